# revision 11
# baseline (speedup 1.0000x reference)
"""Trainium2 Bass kernel for nn_BoothGroupQuant.

Booth/NAF group quantization: q = rne(x*128); NAF-decompose each q into
signed power-of-two digits; per group of 16 consecutive elements keep only
the 8 largest-exponent digits (ties: lower element index first);
reconstruct and scale by 1/128.

Core identity: with t = 3q, u = t ^ q, the NAF nonzero-digit mask of q is u
(digit at exponent e <-> bit e+1), positive digits at u & t, negative at
u & q -- valid directly on two's-complement negatives.  Per-group top-8
selection via int16 SWAR band counters (4 bands of 3 exponents), a halving
tree of grouped adds for band totals, one segmented scan for in-band
ranks, and a packed guard-bit compare.  Design range |q| <= 2730.

v4: asymmetric chunks (512 warmup + 2x1792) to shorten the serial
DMA->ACT head; segment mask DMA'd from host (no strided memsets);
pre-scan group logic merged across chunks, post-scan per chunk; exclusive
rank subtract folded into the guard bias (inclusive compare, bias 0x4210);
tensor_reduce replaced by 2x-rate halving-tree adds; final output sliced
to overlap the tail DMA.
"""
import os
import sys

import numpy as np

for _p in ("/opt/trn_rl_repo", "/root/.axon_site/_ro/trn_rl_repo"):
    if os.path.isdir(_p) and _p not in sys.path:
        sys.path.insert(0, _p)

import concourse.bacc as bacc
import concourse.mybir as mybir
from concourse import bass_utils
from concourse.tile import TileContext

N_CORES = 8
FULL_SHAPE = (4, 1024, 32, 32)
N_TOTAL = 4 * 1024 * 32 * 32          # 4194304
N_CORE = N_TOTAL // N_CORES           # 524288
P = 128                               # SBUF partitions
F_TOTAL = N_CORE // P                 # 4096 free elems per partition
CHUNKS = (512, 1792, 1792)
FMAX = max(CHUNKS)
GT = F_TOTAL // 16                    # 256 groups per partition
SF = 0.0078125

i16 = mybir.dt.int16
f32 = mybir.dt.float32
Alu = mybir.AluOpType
Act = mybir.ActivationFunctionType

_CACHE = {}


def _build():
    nc = bacc.Bacc("TRN2")
    x_in = nc.dram_tensor("x", [P, F_TOTAL], f32, kind="ExternalInput")
    seg_in = nc.dram_tensor("seg", [P, FMAX], i16, kind="ExternalInput")
    y_out = nc.dram_tensor("y", [P, F_TOTAL], f32, kind="ExternalOutput")
    V, S = nc.vector, nc.scalar
    NCH = len(CHUNKS)
    offs = [sum(CHUNKS[:i]) for i in range(NCH)]
    goffs = [o // 16 for o in offs]

    def grp(ap):
        return ap.rearrange("p (g s) -> p g s", s=16)

    with TileContext(nc) as tc:
        with tc.tile_pool(name="const", bufs=1) as cpool:
            seg = cpool.tile([P, FMAX], i16)

            with tc.tile_pool(name="work", bufs=1) as pool:
                def full(nm, c, dt=i16, nb=1):
                    return pool.tile([P, CHUNKS[c]], dt, name=nm, tag=nm,
                                     bufs=nb)

                def tiny(nm, width=GT):
                    return pool.tile([P, width], i16, name=nm, tag=nm)

                q_ = [full(f"q{c}", c) for c in range(NCH)]
                u_ = [full(f"u{c}", c) for c in range(NCH)]
                w_ = [full(f"w{c}", c) for c in range(NCH)]
                Pm_ = [full(f"Pm{c}", c) for c in range(NCH)]
                R2all = pool.tile([P, 2 * GT], i16, name="R2all", tag="R2all")
                R2v = R2all.rearrange("p (k g) -> p k g", k=2)

                # ---------- stage 1: input + q/t for all chunks ----------
                t_ = []
                for c in range(NCH):
                    fc = CHUNKS[c]
                    sl = slice(offs[c], offs[c] + fc)
                    q = q_[c]
                    xt = full("xt", c, f32, nb=3)
                    nc.sync.dma_start(out=xt, in_=x_in[:, sl])
                    if c == 0:
                        nc.sync.dma_start(out=seg, in_=seg_in[:, :])
                    S.activation(q, xt, Act.Copy, scale=128.0)
                    t = full("t", c, nb=3)
                    if c == 1:
                        V.tensor_scalar(t, q, 3, None, Alu.mult)
                    else:
                        S.activation(t, q, Act.Copy, scale=3.0)
                    t_.append(t)

                # ---------- stage 1b per chunk: u + band-count tree ----------
                for c in range(NCH):
                    fc, gc, go = CHUNKS[c], CHUNKS[c] // 16, goffs[c]
                    q, u = q_[c], u_[c]
                    V.tensor_tensor(u, t_[c], q, Alu.bitwise_xor)

                    A = full("A", c, nb=2)
                    V.tensor_scalar(A, u, 1, 0x249,
                                    Alu.logical_shift_right, Alu.bitwise_and)
                    B = full("B", c, nb=2)
                    V.tensor_scalar(B, u, 2, 0x249,
                                    Alu.logical_shift_right, Alu.bitwise_and)
                    C = full("C", c, nb=2)
                    V.tensor_scalar(C, u, 3, 0x249,
                                    Alu.logical_shift_right, Alu.bitwise_and)
                    V.tensor_tensor(A, A, B, Alu.add)
                    V.tensor_tensor(A, A, C, Alu.add)

                    Ag = grp(A)                              # [P, gc, 16]
                    A8 = pool.tile([P, gc * 8], i16, name="A8", tag="A8",
                                   bufs=2)
                    A8v = A8.rearrange("p (g s) -> p g s", s=8)
                    V.tensor_tensor(A8v, Ag[:, :, 0:8], Ag[:, :, 8:16],
                                    Alu.add)                 # fields <= 6
                    D = pool.tile([P, 2 * gc * 8], i16, name="D", tag="D",
                                  bufs=2)
                    Dv = D.rearrange("p (k g s) -> p k g s", k=2, s=8)
                    V.tensor_scalar(Dv[:, 0], A8v, 0x1C7, None,
                                    Alu.bitwise_and)
                    V.tensor_scalar(Dv[:, 1], A8v, 3, 0x1C7,
                                    Alu.logical_shift_right, Alu.bitwise_and)
                    E = pool.tile([P, 2 * gc * 4], i16, name="E", tag="E",
                                  bufs=2)
                    Ev = E.rearrange("p (k g s) -> p k g s", k=2, s=4)
                    V.tensor_tensor(Ev, Dv[:, :, :, 0:4], Dv[:, :, :, 4:8],
                                    Alu.add)
                    F2 = pool.tile([P, 2 * gc * 2], i16, name="F2", tag="F2",
                                   bufs=2)
                    F2v = F2.rearrange("p (k g s) -> p k g s", k=2, s=2)
                    V.tensor_tensor(F2v, Ev[:, :, :, 0:2], Ev[:, :, :, 2:4],
                                    Alu.add)
                    V.tensor_tensor(R2v[:, :, go:go + gc],
                                    F2v[:, :, :, 0], F2v[:, :, :, 1], Alu.add)

                # ---------- merged pre-scan group logic: theta + smx --------
                RE = R2v[:, 0, :]          # S0 + 64*S2 per group
                RO = R2v[:, 1, :]          # S1 + 64*S3
                B2 = tiny("B2")
                V.tensor_scalar(B2, RE, 6, None, Alu.logical_shift_right)
                B1 = tiny("B1")
                V.tensor_scalar(B1, RO, 63, None, Alu.bitwise_and)
                B3 = tiny("B3")
                V.tensor_scalar(B3, RO, 6, None, Alu.logical_shift_right)
                s2 = tiny("s2")
                V.tensor_tensor(s2, B3, B2, Alu.add)
                s1 = tiny("s1")
                V.tensor_tensor(s1, s2, B1, Alu.add)
                # m_b = (suffix_b < 8); bstar = 3 - (m1+m2+m3)
                m3 = tiny("m3")
                V.tensor_scalar(m3, B3, 8, None, Alu.is_lt)
                m2 = tiny("m2")
                V.tensor_scalar(m2, s2, 8, None, Alu.is_lt)
                m1 = tiny("m1")
                V.tensor_scalar(m1, s1, 8, None, Alu.is_lt)
                smx = tiny("smx")
                V.tensor_tensor(smx, m3, m2, Alu.add)
                V.tensor_tensor(smx, smx, m1, Alu.add)
                # Cab = B3*m3 + B2*m2 + B1*m1 ; theta = 8 - Cab in [1, 8]
                V.tensor_tensor(m3, B3, m3, Alu.mult)
                V.tensor_tensor(m2, B2, m2, Alu.mult)
                V.tensor_tensor(m1, B1, m1, Alu.mult)
                V.tensor_tensor(m3, m3, m2, Alu.add)
                V.tensor_tensor(m3, m3, m1, Alu.add)
                theta = tiny("theta")
                V.tensor_scalar(theta, m3, -1, 8, Alu.mult, Alu.add)
                t2s = tiny("t2s")
                V.tensor_scalar(t2s, theta, 1024, None, Alu.mult)

                # ---------- stage 2: shift + spread + scan (interleaved) ----
                amtx_of = []
                for c in range(NCH):
                    gc, go = CHUNKS[c] // 16, goffs[c]
                    amtx = full("amtx", c, nb=3)
                    amtx_of.append(amtx)
                    # amt = 3*bstar + 1 = 10 - 3*(m1+m2+m3)
                    S.activation(
                        grp(amtx),
                        smx[:, go:go + gc, None].broadcast_to((P, gc, 16)),
                        Act.Copy, scale=-3.0, bias=10.0)
                sp_of, sm_of = [], []
                for c in range(NCH):
                    w = w_[c]
                    V.tensor_tensor(w, u_[c], amtx_of[c],
                                    Alu.logical_shift_right)
                    sp = full("sp", c, nb=2)
                    V.tensor_scalar(sp, w, 7, None, Alu.bitwise_and)
                    sm = full("sm", c, nb=2)
                    S.activation(sm, sp, Act.Copy, scale=float(0x111))
                    sm_of.append(sm)
                for c in range(NCH):
                    fc = CHUNKS[c]
                    s = full("s", c, nb=2)
                    V.tensor_scalar(s, sm_of[c], 0x421, None, Alu.bitwise_and)
                    V.tensor_tensor_scan(Pm_[c], seg[:, 0:fc], s, 0.0,
                                         Alu.mult, Alu.add)

                # ---------- stage 3a per chunk: thresholds + thx ------------
                thx_of = []
                for c in range(NCH):
                    fc, gc, go = CHUNKS[c], CHUNKS[c] // 16, goffs[c]
                    gsl = slice(go, go + gc)
                    Pm = Pm_[c]
                    TPv = grp(Pm)[:, :, 15]
                    n2 = tiny("n2", gc)
                    V.tensor_scalar(n2, TPv, 10, 31,
                                    Alu.logical_shift_right, Alu.bitwise_and)
                    n1 = tiny("n1", gc)
                    V.tensor_scalar(n1, TPv, 5, 31,
                                    Alu.logical_shift_right, Alu.bitwise_and)
                    th1 = tiny("th1", gc)
                    V.tensor_tensor(th1, theta[:, gsl], n2, Alu.subtract)
                    th0 = tiny("th0", gc)
                    V.tensor_tensor(th0, th1, n1, Alu.subtract)
                    th1c = tiny("th1c", gc)
                    V.tensor_scalar(th1c, th1, 0, 32, Alu.max, Alu.mult)
                    th0c = tiny("th0c", gc)
                    V.tensor_scalar(th0c, th0, 0, None, Alu.max)
                    V.tensor_tensor(th0c, th0c, th1c, Alu.add)
                    V.tensor_tensor(th0c, th0c, t2s[:, gsl], Alu.add)

                    # inclusive-rank compare: bias 0x4210 (+1/field vs the
                    # exclusive form); non-digit guards masked by w below
                    thx = full("thx", c, nb=3)
                    S.activation(
                        grp(thx),
                        th0c[:, :, None].broadcast_to((P, gc, 16)),
                        Act.Copy, bias=float(0x4210))
                    thx_of.append(thx)

                # ---------- stage 3b per chunk: compare + reconstruct -------
                for c in range(NCH):
                    fc, gc, go = CHUNKS[c], CHUNKS[c] // 16, goffs[c]
                    q, u, w, Pm = q_[c], u_[c], w_[c], Pm_[c]
                    thx = thx_of[c]
                    X = full("X", c, nb=2)
                    V.tensor_tensor(X, thx, Pm, Alu.subtract)
                    # gather guard bits {4,9,14} -> keep mask at bits {0,1,2}
                    k1 = full("k1", c, nb=2)
                    V.tensor_scalar(k1, X, 12, 4,
                                    Alu.logical_shift_right, Alu.bitwise_and)
                    k2 = full("k2", c, nb=2)
                    V.tensor_scalar(k2, X, 4, 0x21,
                                    Alu.logical_shift_right, Alu.bitwise_and)
                    k3 = full("k3", c, nb=2)
                    V.tensor_scalar(k3, k2, 0x11, None, Alu.mult)
                    V.tensor_scalar(k3, k3, 4, -8,
                                    Alu.logical_shift_right, Alu.bitwise_or)
                    V.tensor_tensor(k1, k1, k3, Alu.bitwise_or)   # Kband
                    V.tensor_tensor(w, w, k1, Alu.bitwise_and)    # wk
                    V.tensor_tensor(w, w, amtx_of[c],
                                    Alu.logical_shift_left)       # UK
                    # val = UK - 2*(UK & q)
                    V.tensor_tensor(q, w, q, Alu.bitwise_and)     # NM
                    NM2 = full("NM2", c, nb=2)
                    if c == NCH - 1:
                        V.tensor_scalar(NM2, q, 1, None,
                                        Alu.logical_shift_left)
                    else:
                        S.activation(NM2, q, Act.Copy, scale=2.0)
                    V.tensor_tensor(w, w, NM2, Alu.subtract)      # val

                    yt = full("yt", c, f32, nb=2)
                    nsl = 4 if c == NCH - 1 else 1
                    step = fc // nsl
                    for k in range(nsl):
                        ksl = slice(k * step, (k + 1) * step)
                        S.activation(yt[:, ksl], w[:, ksl], Act.Copy,
                                     scale=SF / 2.0)
                        nc.sync.dma_start(
                            out=y_out[:, offs[c] + k * step:
                                      offs[c] + (k + 1) * step],
                            in_=yt[:, ksl])

    nc.compile()
    return nc


def _get_nc():
    if "nc" not in _CACHE:
        _CACHE["nc"] = _build()
    return _CACHE["nc"]


def _seg_np():
    one_group = np.array([0] + [1] * 15, dtype=np.int16)
    row = np.tile(one_group, FMAX // 16)
    return np.broadcast_to(row, (P, FMAX)).copy()


def kernel(x: np.ndarray, _trace: bool = False, _trace_kwargs=None):
    assert x.shape == FULL_SHAPE and x.dtype == np.float32, (x.shape, x.dtype)
    nc = _get_nc()
    flat = np.ascontiguousarray(x).reshape(N_CORES, P, F_TOTAL)
    seg = _seg_np()
    in_maps = [{"x": flat[i], "seg": seg} for i in range(N_CORES)]
    kw = {}
    if _trace:
        kw = {"trace": True, **(_trace_kwargs or {})}
    res = bass_utils.run_bass_kernel_spmd(
        nc, in_maps, core_ids=list(range(N_CORES)), **kw)
    out = np.stack([res.results[i]["y"] for i in range(N_CORES)], axis=0)
    out = out.reshape(FULL_SHAPE).astype(np.float32)
    if _trace:
        return out, res
    return out


# revision 12
# speedup vs baseline: 1.1367x; 1.1367x over previous
"""Trainium2 Bass kernel for nn_BoothGroupQuant.

Booth/NAF group quantization: q = rne(x*128); NAF-decompose each q into
signed power-of-two digits; per group of 16 consecutive elements keep only
the 8 largest-exponent digits (ties: lower element index first);
reconstruct and scale by 1/128.

Core identity: with t = 3q, u = t ^ q, the NAF nonzero-digit mask of q is u
(digit at exponent e <-> bit e+1), positive digits at u & t, negative at
u & q -- valid directly on two's-complement negatives.  Per-group top-8
selection via int16 SWAR band counters (4 bands of 3 exponents), a halving
tree of grouped adds for band totals, one segmented scan for in-band
ranks, and a packed guard-bit compare.  Design range |q| <= 2730.

v4: asymmetric chunks (512 warmup + 2x1792) to shorten the serial
DMA->ACT head; segment mask DMA'd from host (no strided memsets);
pre-scan group logic merged across chunks, post-scan per chunk; exclusive
rank subtract folded into the guard bias (inclusive compare, bias 0x4210);
tensor_reduce replaced by 2x-rate halving-tree adds; final output sliced
to overlap the tail DMA.
"""
import os
import sys

import numpy as np

for _p in ("/opt/trn_rl_repo", "/root/.axon_site/_ro/trn_rl_repo"):
    if os.path.isdir(_p) and _p not in sys.path:
        sys.path.insert(0, _p)

import concourse.bacc as bacc
import concourse.mybir as mybir
from concourse import bass_utils
from concourse.tile import TileContext

N_CORES = 8
FULL_SHAPE = (4, 1024, 32, 32)
N_TOTAL = 4 * 1024 * 32 * 32          # 4194304
N_CORE = N_TOTAL // N_CORES           # 524288
P = 128                               # SBUF partitions
F_TOTAL = N_CORE // P                 # 4096 free elems per partition
CHUNKS = (512, 1792, 1792)
FMAX = max(CHUNKS)
GT = F_TOTAL // 16                    # 256 groups per partition
SF = 0.0078125

i16 = mybir.dt.int16
f32 = mybir.dt.float32
Alu = mybir.AluOpType
Act = mybir.ActivationFunctionType

_CACHE = {}


def _build():
    nc = bacc.Bacc("TRN2")
    x_in = nc.dram_tensor("x", [P, F_TOTAL], f32, kind="ExternalInput")
    seg_in = nc.dram_tensor("seg", [P, FMAX], i16, kind="ExternalInput")
    y_out = nc.dram_tensor("y", [P, F_TOTAL], f32, kind="ExternalOutput")
    V, S = nc.vector, nc.scalar
    NCH = len(CHUNKS)
    offs = [sum(CHUNKS[:i]) for i in range(NCH)]
    goffs = [o // 16 for o in offs]

    def grp(ap):
        return ap.rearrange("p (g s) -> p g s", s=16)

    with TileContext(nc) as tc:
        with tc.tile_pool(name="const", bufs=1) as cpool:
            seg = cpool.tile([P, FMAX], i16)

            with tc.tile_pool(name="work", bufs=1) as pool:
                def full(nm, c, dt=i16, nb=1):
                    return pool.tile([P, CHUNKS[c]], dt, name=nm, tag=nm,
                                     bufs=nb)

                def tiny(nm, width=GT):
                    return pool.tile([P, width], i16, name=nm, tag=nm)

                q_ = [full(f"q{c}", c) for c in range(NCH)]
                u_ = [full(f"u{c}", c) for c in range(NCH)]
                w_ = [full(f"w{c}", c) for c in range(NCH)]
                Pm_ = [full(f"Pm{c}", c) for c in range(NCH)]
                R2all = pool.tile([P, 2 * GT], i16, name="R2all", tag="R2all")
                R2v = R2all.rearrange("p (k g) -> p k g", k=2)

                # ---------- stage 1: input + q/t for all chunks ----------
                t_ = []
                for c in range(NCH):
                    fc = CHUNKS[c]
                    sl = slice(offs[c], offs[c] + fc)
                    q = q_[c]
                    xt = full("xt", c, f32, nb=3)
                    nc.sync.dma_start(out=xt, in_=x_in[:, sl])
                    if c == 0:
                        nc.sync.dma_start(out=seg, in_=seg_in[:, :])
                    S.activation(q, xt, Act.Copy, scale=128.0)
                    t = full("t", c, nb=3)
                    S.activation(t, q, Act.Copy, scale=3.0)
                    t_.append(t)

                # ---------- stage 1b per chunk: u + band-count tree ----------
                for c in range(NCH):
                    fc, gc, go = CHUNKS[c], CHUNKS[c] // 16, goffs[c]
                    q, u = q_[c], u_[c]
                    V.tensor_tensor(u, t_[c], q, Alu.bitwise_xor)

                    A = full("A", c, nb=2)
                    V.tensor_scalar(A, u, 1, 0x249,
                                    Alu.logical_shift_right, Alu.bitwise_and)
                    B = full("B", c, nb=2)
                    V.tensor_scalar(B, u, 2, 0x249,
                                    Alu.logical_shift_right, Alu.bitwise_and)
                    C = full("C", c, nb=2)
                    V.tensor_scalar(C, u, 3, 0x249,
                                    Alu.logical_shift_right, Alu.bitwise_and)
                    V.tensor_tensor(A, A, B, Alu.add)
                    V.tensor_tensor(A, A, C, Alu.add)

                    Ag = grp(A)                              # [P, gc, 16]
                    A8 = pool.tile([P, gc * 8], i16, name="A8", tag="A8",
                                   bufs=2)
                    A8v = A8.rearrange("p (g s) -> p g s", s=8)
                    V.tensor_tensor(A8v, Ag[:, :, 0:8], Ag[:, :, 8:16],
                                    Alu.add)                 # fields <= 6
                    D = pool.tile([P, 2 * gc * 8], i16, name="D", tag="D",
                                  bufs=2)
                    Dv = D.rearrange("p (k g s) -> p k g s", k=2, s=8)
                    V.tensor_scalar(Dv[:, 0], A8v, 0x1C7, None,
                                    Alu.bitwise_and)
                    V.tensor_scalar(Dv[:, 1], A8v, 3, 0x1C7,
                                    Alu.logical_shift_right, Alu.bitwise_and)
                    E = pool.tile([P, 2 * gc * 4], i16, name="E", tag="E",
                                  bufs=2)
                    Ev = E.rearrange("p (k g s) -> p k g s", k=2, s=4)
                    V.tensor_tensor(Ev, Dv[:, :, :, 0:4], Dv[:, :, :, 4:8],
                                    Alu.add)
                    F2 = pool.tile([P, 2 * gc * 2], i16, name="F2", tag="F2",
                                   bufs=2)
                    F2v = F2.rearrange("p (k g s) -> p k g s", k=2, s=2)
                    V.tensor_tensor(F2v, Ev[:, :, :, 0:2], Ev[:, :, :, 2:4],
                                    Alu.add)
                    V.tensor_tensor(R2v[:, :, go:go + gc],
                                    F2v[:, :, :, 0], F2v[:, :, :, 1], Alu.add)

                # ---------- merged pre-scan group logic: theta + smx --------
                RE = R2v[:, 0, :]          # S0 + 64*S2 per group
                RO = R2v[:, 1, :]          # S1 + 64*S3
                B2 = tiny("B2")
                V.tensor_scalar(B2, RE, 6, None, Alu.logical_shift_right)
                B1 = tiny("B1")
                V.tensor_scalar(B1, RO, 63, None, Alu.bitwise_and)
                B3 = tiny("B3")
                V.tensor_scalar(B3, RO, 6, None, Alu.logical_shift_right)
                s2 = tiny("s2")
                V.tensor_tensor(s2, B3, B2, Alu.add)
                s1 = tiny("s1")
                V.tensor_tensor(s1, s2, B1, Alu.add)
                # m_b = (suffix_b < 8); bstar = 3 - (m1+m2+m3)
                m3 = tiny("m3")
                V.tensor_scalar(m3, B3, 8, None, Alu.is_lt)
                m2 = tiny("m2")
                V.tensor_scalar(m2, s2, 8, None, Alu.is_lt)
                m1 = tiny("m1")
                V.tensor_scalar(m1, s1, 8, None, Alu.is_lt)
                smx = tiny("smx")
                V.tensor_tensor(smx, m3, m2, Alu.add)
                V.tensor_tensor(smx, smx, m1, Alu.add)
                # Cab = B3*m3 + B2*m2 + B1*m1 ; theta = 8 - Cab in [1, 8]
                V.tensor_tensor(m3, B3, m3, Alu.mult)
                V.tensor_tensor(m2, B2, m2, Alu.mult)
                V.tensor_tensor(m1, B1, m1, Alu.mult)
                V.tensor_tensor(m3, m3, m2, Alu.add)
                V.tensor_tensor(m3, m3, m1, Alu.add)
                theta = tiny("theta")
                V.tensor_scalar(theta, m3, -1, 8, Alu.mult, Alu.add)
                t2s = tiny("t2s")
                V.tensor_scalar(t2s, theta, 1024, None, Alu.mult)

                # ---------- stage 2: shift + spread + scan (interleaved) ----
                amtx_of = []
                for c in range(NCH):
                    gc, go = CHUNKS[c] // 16, goffs[c]
                    amtx = full("amtx", c, nb=3)
                    amtx_of.append(amtx)
                    # amt = 3*bstar + 1 = 10 - 3*(m1+m2+m3)
                    S.activation(
                        grp(amtx),
                        smx[:, go:go + gc, None].broadcast_to((P, gc, 16)),
                        Act.Copy, scale=-3.0, bias=10.0)
                sp_of, sm_of = [], []
                for c in range(NCH):
                    w = w_[c]
                    V.tensor_tensor(w, u_[c], amtx_of[c],
                                    Alu.logical_shift_right)
                    sp = full("sp", c, nb=2)
                    V.tensor_scalar(sp, w, 7, None, Alu.bitwise_and)
                    sm = full("sm", c, nb=2)
                    S.activation(sm, sp, Act.Copy, scale=float(0x111))
                    sm_of.append(sm)
                for c in range(NCH):
                    fc = CHUNKS[c]
                    s = full("s", c, nb=2)
                    V.tensor_scalar(s, sm_of[c], 0x421, None, Alu.bitwise_and)
                    V.tensor_tensor_scan(Pm_[c], seg[:, 0:fc], s, 0.0,
                                         Alu.mult, Alu.add)

                # ---------- stage 3a per chunk: thresholds + thx ------------
                thx_of = []
                for c in range(NCH):
                    fc, gc, go = CHUNKS[c], CHUNKS[c] // 16, goffs[c]
                    gsl = slice(go, go + gc)
                    Pm = Pm_[c]
                    TPv = grp(Pm)[:, :, 15]
                    n2 = tiny("n2", gc)
                    V.tensor_scalar(n2, TPv, 10, 31,
                                    Alu.logical_shift_right, Alu.bitwise_and)
                    n1 = tiny("n1", gc)
                    V.tensor_scalar(n1, TPv, 5, 31,
                                    Alu.logical_shift_right, Alu.bitwise_and)
                    th1 = tiny("th1", gc)
                    V.tensor_tensor(th1, theta[:, gsl], n2, Alu.subtract)
                    th0 = tiny("th0", gc)
                    V.tensor_tensor(th0, th1, n1, Alu.subtract)
                    th1c = tiny("th1c", gc)
                    V.tensor_scalar(th1c, th1, 0, 32, Alu.max, Alu.mult)
                    th0c = tiny("th0c", gc)
                    V.tensor_scalar(th0c, th0, 0, None, Alu.max)
                    V.tensor_tensor(th0c, th0c, th1c, Alu.add)
                    V.tensor_tensor(th0c, th0c, t2s[:, gsl], Alu.add)

                    # inclusive-rank compare: bias 0x4210 (+1/field vs the
                    # exclusive form); non-digit guards masked by w below
                    thx = full("thx", c, nb=3)
                    S.activation(
                        grp(thx),
                        th0c[:, :, None].broadcast_to((P, gc, 16)),
                        Act.Copy, bias=float(0x4210))
                    thx_of.append(thx)

                # ---------- stage 3b per chunk: compare + reconstruct -------
                for c in range(NCH):
                    fc, gc, go = CHUNKS[c], CHUNKS[c] // 16, goffs[c]
                    q, u, w, Pm = q_[c], u_[c], w_[c], Pm_[c]
                    thx = thx_of[c]
                    X = full("X", c, nb=2)
                    V.tensor_tensor(X, thx, Pm, Alu.subtract)
                    # gather guard bits {4,9,14} -> keep mask at bits {0,1,2}
                    k1 = full("k1", c, nb=2)
                    V.tensor_scalar(k1, X, 12, 4,
                                    Alu.logical_shift_right, Alu.bitwise_and)
                    k2 = full("k2", c, nb=2)
                    V.tensor_scalar(k2, X, 4, 0x21,
                                    Alu.logical_shift_right, Alu.bitwise_and)
                    k3 = full("k3", c, nb=2)
                    V.tensor_scalar(k3, k2, 0x11, None, Alu.mult)
                    V.tensor_scalar(k3, k3, 4, -8,
                                    Alu.logical_shift_right, Alu.bitwise_or)
                    V.tensor_tensor(k1, k1, k3, Alu.bitwise_or)   # Kband
                    V.tensor_tensor(w, w, k1, Alu.bitwise_and)    # wk
                    V.tensor_tensor(w, w, amtx_of[c],
                                    Alu.logical_shift_left)       # UK
                    # val = UK - 2*(UK & q)
                    V.tensor_tensor(q, w, q, Alu.bitwise_and)     # NM
                    NM2 = full("NM2", c, nb=2)
                    if c == NCH - 1:
                        V.tensor_scalar(NM2, q, 1, None,
                                        Alu.logical_shift_left)
                    else:
                        S.activation(NM2, q, Act.Copy, scale=2.0)
                    V.tensor_tensor(w, w, NM2, Alu.subtract)      # val

                    yt = full("yt", c, f32, nb=2)
                    nsl = 4 if c == NCH - 1 else 1
                    step = fc // nsl
                    for k in range(nsl):
                        ksl = slice(k * step, (k + 1) * step)
                        S.activation(yt[:, ksl], w[:, ksl], Act.Copy,
                                     scale=SF / 2.0)
                        nc.sync.dma_start(
                            out=y_out[:, offs[c] + k * step:
                                      offs[c] + (k + 1) * step],
                            in_=yt[:, ksl])

    nc.compile()
    return nc


def _get_nc():
    if "nc" not in _CACHE:
        _CACHE["nc"] = _build()
    return _CACHE["nc"]


def _seg_np():
    one_group = np.array([0] + [1] * 15, dtype=np.int16)
    row = np.tile(one_group, FMAX // 16)
    return np.broadcast_to(row, (P, FMAX)).copy()


def kernel(x: np.ndarray, _trace: bool = False, _trace_kwargs=None):
    assert x.shape == FULL_SHAPE and x.dtype == np.float32, (x.shape, x.dtype)
    nc = _get_nc()
    flat = np.ascontiguousarray(x).reshape(N_CORES, P, F_TOTAL)
    seg = _seg_np()
    in_maps = [{"x": flat[i], "seg": seg} for i in range(N_CORES)]
    kw = {}
    if _trace:
        kw = {"trace": True, **(_trace_kwargs or {})}
    res = bass_utils.run_bass_kernel_spmd(
        nc, in_maps, core_ids=list(range(N_CORES)), **kw)
    out = np.stack([res.results[i]["y"] for i in range(N_CORES)], axis=0)
    out = out.reshape(FULL_SHAPE).astype(np.float32)
    if _trace:
        return out, res
    return out


# revision 13
# speedup vs baseline: 1.1821x; 1.0399x over previous
"""Trainium2 Bass kernel for nn_BoothGroupQuant.

Booth/NAF group quantization: q = rne(x*128); NAF-decompose each q into
signed power-of-two digits; per group of 16 consecutive elements keep only
the 8 largest-exponent digits (ties: lower element index first);
reconstruct and scale by 1/128.

Core identity: with t = 3q, u = t ^ q, the NAF nonzero-digit mask of q is u
(digit at exponent e <-> bit e+1), positive digits at u & t, negative at
u & q -- valid directly on two's-complement negatives.  Per-group top-8
selection via int16 SWAR band counters (4 bands of 3 exponents), a halving
tree of grouped adds for band totals, one segmented scan for in-band
ranks, and a packed guard-bit compare.  Design range |q| <= 2730.

v4: asymmetric chunks (512 warmup + 2x1792) to shorten the serial
DMA->ACT head; segment mask DMA'd from host (no strided memsets);
pre-scan group logic merged across chunks, post-scan per chunk; exclusive
rank subtract folded into the guard bias (inclusive compare, bias 0x4210);
tensor_reduce replaced by 2x-rate halving-tree adds; final output sliced
to overlap the tail DMA.
"""
import os
import sys

import numpy as np

for _p in ("/opt/trn_rl_repo", "/root/.axon_site/_ro/trn_rl_repo"):
    if os.path.isdir(_p) and _p not in sys.path:
        sys.path.insert(0, _p)

import concourse.bacc as bacc
import concourse.mybir as mybir
from concourse import bass_utils
from concourse.tile import TileContext

N_CORES = 8
FULL_SHAPE = (4, 1024, 32, 32)
N_TOTAL = 4 * 1024 * 32 * 32          # 4194304
N_CORE = N_TOTAL // N_CORES           # 524288
P = 128                               # SBUF partitions
F_TOTAL = N_CORE // P                 # 4096 free elems per partition
CHUNKS = (768, 1664, 1664)
FMAX = max(CHUNKS)
GT = F_TOTAL // 16                    # 256 groups per partition
SF = 0.0078125

i16 = mybir.dt.int16
f32 = mybir.dt.float32
Alu = mybir.AluOpType
Act = mybir.ActivationFunctionType

_CACHE = {}


def _build():
    nc = bacc.Bacc("TRN2")
    x_in = nc.dram_tensor("x", [P, F_TOTAL], f32, kind="ExternalInput")
    seg_in = nc.dram_tensor("seg", [P, FMAX], i16, kind="ExternalInput")
    y_out = nc.dram_tensor("y", [P, F_TOTAL], f32, kind="ExternalOutput")
    V, S = nc.vector, nc.scalar
    NCH = len(CHUNKS)
    offs = [sum(CHUNKS[:i]) for i in range(NCH)]
    goffs = [o // 16 for o in offs]

    def grp(ap):
        return ap.rearrange("p (g s) -> p g s", s=16)

    with TileContext(nc) as tc:
        with tc.tile_pool(name="const", bufs=1) as cpool:
            seg = cpool.tile([P, FMAX], i16)

            with tc.tile_pool(name="work", bufs=1) as pool:
                def full(nm, c, dt=i16, nb=1):
                    return pool.tile([P, CHUNKS[c]], dt, name=nm, tag=nm,
                                     bufs=nb)

                def tiny(nm, width=GT):
                    return pool.tile([P, width], i16, name=nm, tag=nm)

                q_ = [full(f"q{c}", c) for c in range(NCH)]
                u_ = [full(f"u{c}", c) for c in range(NCH)]
                w_ = [full(f"w{c}", c) for c in range(NCH)]
                Pm_ = [full(f"Pm{c}", c) for c in range(NCH)]
                R2all = pool.tile([P, 2 * GT], i16, name="R2all", tag="R2all")
                R2v = R2all.rearrange("p (k g) -> p k g", k=2)

                # ---------- stage 1: input + q/t for all chunks ----------
                t_ = []
                for c in range(NCH):
                    fc = CHUNKS[c]
                    sl = slice(offs[c], offs[c] + fc)
                    q = q_[c]
                    xt = full("xt", c, f32, nb=3)
                    nc.sync.dma_start(out=xt, in_=x_in[:, sl])
                    if c == 0:
                        nc.sync.dma_start(out=seg, in_=seg_in[:, :])
                    S.activation(q, xt, Act.Copy, scale=128.0)
                    t = full("t", c, nb=3)
                    S.activation(t, q, Act.Copy, scale=3.0)
                    t_.append(t)

                # ---------- stage 1b per chunk: u + band-count tree ----------
                for c in range(NCH):
                    fc, gc, go = CHUNKS[c], CHUNKS[c] // 16, goffs[c]
                    q, u = q_[c], u_[c]
                    V.tensor_tensor(u, t_[c], q, Alu.bitwise_xor)

                    A = full("A", c, nb=2)
                    V.tensor_scalar(A, u, 1, 0x249,
                                    Alu.logical_shift_right, Alu.bitwise_and)
                    B = full("B", c, nb=2)
                    V.tensor_scalar(B, u, 2, 0x249,
                                    Alu.logical_shift_right, Alu.bitwise_and)
                    C = full("C", c, nb=2)
                    V.tensor_scalar(C, u, 3, 0x249,
                                    Alu.logical_shift_right, Alu.bitwise_and)
                    V.tensor_tensor(A, A, B, Alu.add)
                    V.tensor_tensor(A, A, C, Alu.add)

                    Ag = grp(A)                              # [P, gc, 16]
                    A8 = pool.tile([P, gc * 8], i16, name="A8", tag="A8",
                                   bufs=2)
                    A8v = A8.rearrange("p (g s) -> p g s", s=8)
                    V.tensor_tensor(A8v, Ag[:, :, 0:8], Ag[:, :, 8:16],
                                    Alu.add)                 # fields <= 6
                    D = pool.tile([P, 2 * gc * 8], i16, name="D", tag="D",
                                  bufs=2)
                    Dv = D.rearrange("p (k g s) -> p k g s", k=2, s=8)
                    V.tensor_scalar(Dv[:, 0], A8v, 0x1C7, None,
                                    Alu.bitwise_and)
                    V.tensor_scalar(Dv[:, 1], A8v, 3, 0x1C7,
                                    Alu.logical_shift_right, Alu.bitwise_and)
                    E = pool.tile([P, 2 * gc * 4], i16, name="E", tag="E",
                                  bufs=2)
                    Ev = E.rearrange("p (k g s) -> p k g s", k=2, s=4)
                    V.tensor_tensor(Ev, Dv[:, :, :, 0:4], Dv[:, :, :, 4:8],
                                    Alu.add)
                    F2 = pool.tile([P, 2 * gc * 2], i16, name="F2", tag="F2",
                                   bufs=2)
                    F2v = F2.rearrange("p (k g s) -> p k g s", k=2, s=2)
                    V.tensor_tensor(F2v, Ev[:, :, :, 0:2], Ev[:, :, :, 2:4],
                                    Alu.add)
                    V.tensor_tensor(R2v[:, :, go:go + gc],
                                    F2v[:, :, :, 0], F2v[:, :, :, 1], Alu.add)

                # ---------- merged pre-scan group logic: theta + smx --------
                RE = R2v[:, 0, :]          # S0 + 64*S2 per group
                RO = R2v[:, 1, :]          # S1 + 64*S3
                B2 = tiny("B2")
                V.tensor_scalar(B2, RE, 6, None, Alu.logical_shift_right)
                B1 = tiny("B1")
                V.tensor_scalar(B1, RO, 63, None, Alu.bitwise_and)
                B3 = tiny("B3")
                V.tensor_scalar(B3, RO, 6, None, Alu.logical_shift_right)
                s2 = tiny("s2")
                V.tensor_tensor(s2, B3, B2, Alu.add)
                s1 = tiny("s1")
                V.tensor_tensor(s1, s2, B1, Alu.add)
                # m_b = (suffix_b < 8); bstar = 3 - (m1+m2+m3)
                m3 = tiny("m3")
                V.tensor_scalar(m3, B3, 8, None, Alu.is_lt)
                m2 = tiny("m2")
                V.tensor_scalar(m2, s2, 8, None, Alu.is_lt)
                m1 = tiny("m1")
                V.tensor_scalar(m1, s1, 8, None, Alu.is_lt)
                smx = tiny("smx")
                V.tensor_tensor(smx, m3, m2, Alu.add)
                V.tensor_tensor(smx, smx, m1, Alu.add)
                # Cab = B3*m3 + B2*m2 + B1*m1 ; theta = 8 - Cab in [1, 8]
                V.tensor_tensor(m3, B3, m3, Alu.mult)
                V.tensor_tensor(m2, B2, m2, Alu.mult)
                V.tensor_tensor(m1, B1, m1, Alu.mult)
                V.tensor_tensor(m3, m3, m2, Alu.add)
                V.tensor_tensor(m3, m3, m1, Alu.add)
                theta = tiny("theta")
                V.tensor_scalar(theta, m3, -1, 8, Alu.mult, Alu.add)
                t2s = tiny("t2s")
                V.tensor_scalar(t2s, theta, 1024, None, Alu.mult)

                # ---------- stage 2: shift + spread + scan (interleaved) ----
                amtx_of = []
                for c in range(NCH):
                    gc, go = CHUNKS[c] // 16, goffs[c]
                    amtx = full("amtx", c, nb=3)
                    amtx_of.append(amtx)
                    # amt = 3*bstar + 1 = 10 - 3*(m1+m2+m3)
                    S.activation(
                        grp(amtx),
                        smx[:, go:go + gc, None].broadcast_to((P, gc, 16)),
                        Act.Copy, scale=-3.0, bias=10.0)
                sp_of, sm_of = [], []
                for c in range(NCH):
                    w = w_[c]
                    V.tensor_tensor(w, u_[c], amtx_of[c],
                                    Alu.logical_shift_right)
                    sp = full("sp", c, nb=2)
                    V.tensor_scalar(sp, w, 7, None, Alu.bitwise_and)
                    sm = full("sm", c, nb=2)
                    S.activation(sm, sp, Act.Copy, scale=float(0x111))
                    sm_of.append(sm)
                for c in range(NCH):
                    fc = CHUNKS[c]
                    s = full("s", c, nb=2)
                    V.tensor_scalar(s, sm_of[c], 0x421, None, Alu.bitwise_and)
                    V.tensor_tensor_scan(Pm_[c], seg[:, 0:fc], s, 0.0,
                                         Alu.mult, Alu.add)

                # ---------- stage 3a per chunk: thresholds + thx ------------
                thx_of = []
                for c in range(NCH):
                    fc, gc, go = CHUNKS[c], CHUNKS[c] // 16, goffs[c]
                    gsl = slice(go, go + gc)
                    Pm = Pm_[c]
                    TPv = grp(Pm)[:, :, 15]
                    n2 = tiny("n2", gc)
                    V.tensor_scalar(n2, TPv, 10, 31,
                                    Alu.logical_shift_right, Alu.bitwise_and)
                    n1 = tiny("n1", gc)
                    V.tensor_scalar(n1, TPv, 5, 31,
                                    Alu.logical_shift_right, Alu.bitwise_and)
                    th1 = tiny("th1", gc)
                    V.tensor_tensor(th1, theta[:, gsl], n2, Alu.subtract)
                    th0 = tiny("th0", gc)
                    V.tensor_tensor(th0, th1, n1, Alu.subtract)
                    th1c = tiny("th1c", gc)
                    V.tensor_scalar(th1c, th1, 0, 32, Alu.max, Alu.mult)
                    th0c = tiny("th0c", gc)
                    V.tensor_scalar(th0c, th0, 0, None, Alu.max)
                    V.tensor_tensor(th0c, th0c, th1c, Alu.add)
                    V.tensor_tensor(th0c, th0c, t2s[:, gsl], Alu.add)

                    # inclusive-rank compare: bias 0x4210 (+1/field vs the
                    # exclusive form); non-digit guards masked by w below
                    thx = full("thx", c, nb=3)
                    S.activation(
                        grp(thx),
                        th0c[:, :, None].broadcast_to((P, gc, 16)),
                        Act.Copy, bias=float(0x4210))
                    thx_of.append(thx)

                # ---------- stage 3b per chunk: compare + reconstruct -------
                for c in range(NCH):
                    fc, gc, go = CHUNKS[c], CHUNKS[c] // 16, goffs[c]
                    q, u, w, Pm = q_[c], u_[c], w_[c], Pm_[c]
                    thx = thx_of[c]
                    X = full("X", c, nb=2)
                    V.tensor_tensor(X, thx, Pm, Alu.subtract)
                    # gather guard bits {4,9,14} -> keep mask at bits {0,1,2}
                    k1 = full("k1", c, nb=2)
                    V.tensor_scalar(k1, X, 12, 4,
                                    Alu.logical_shift_right, Alu.bitwise_and)
                    k2 = full("k2", c, nb=2)
                    V.tensor_scalar(k2, X, 4, 0x21,
                                    Alu.logical_shift_right, Alu.bitwise_and)
                    k3 = full("k3", c, nb=2)
                    V.tensor_scalar(k3, k2, 0x11, None, Alu.mult)
                    V.tensor_scalar(k3, k3, 4, -8,
                                    Alu.logical_shift_right, Alu.bitwise_or)
                    V.tensor_tensor(k1, k1, k3, Alu.bitwise_or)   # Kband
                    V.tensor_tensor(w, w, k1, Alu.bitwise_and)    # wk
                    V.tensor_tensor(w, w, amtx_of[c],
                                    Alu.logical_shift_left)       # UK
                    # val = UK - 2*(UK & q)
                    V.tensor_tensor(q, w, q, Alu.bitwise_and)     # NM
                    NM2 = full("NM2", c, nb=2)
                    if c == NCH - 1:
                        V.tensor_scalar(NM2, q, 1, None,
                                        Alu.logical_shift_left)
                    else:
                        S.activation(NM2, q, Act.Copy, scale=2.0)
                    V.tensor_tensor(w, w, NM2, Alu.subtract)      # val

                    yt = full("yt", c, f32, nb=2)
                    nsl = 4 if c == NCH - 1 else 1
                    step = fc // nsl
                    for k in range(nsl):
                        ksl = slice(k * step, (k + 1) * step)
                        S.activation(yt[:, ksl], w[:, ksl], Act.Copy,
                                     scale=SF / 2.0)
                        nc.sync.dma_start(
                            out=y_out[:, offs[c] + k * step:
                                      offs[c] + (k + 1) * step],
                            in_=yt[:, ksl])

    nc.compile()
    return nc


def _get_nc():
    if "nc" not in _CACHE:
        _CACHE["nc"] = _build()
    return _CACHE["nc"]


def _seg_np():
    one_group = np.array([0] + [1] * 15, dtype=np.int16)
    row = np.tile(one_group, FMAX // 16)
    return np.broadcast_to(row, (P, FMAX)).copy()


def kernel(x: np.ndarray, _trace: bool = False, _trace_kwargs=None):
    assert x.shape == FULL_SHAPE and x.dtype == np.float32, (x.shape, x.dtype)
    nc = _get_nc()
    flat = np.ascontiguousarray(x).reshape(N_CORES, P, F_TOTAL)
    seg = _seg_np()
    in_maps = [{"x": flat[i], "seg": seg} for i in range(N_CORES)]
    kw = {}
    if _trace:
        kw = {"trace": True, **(_trace_kwargs or {})}
    res = bass_utils.run_bass_kernel_spmd(
        nc, in_maps, core_ids=list(range(N_CORES)), **kw)
    out = np.stack([res.results[i]["y"] for i in range(N_CORES)], axis=0)
    out = out.reshape(FULL_SHAPE).astype(np.float32)
    if _trace:
        return out, res
    return out
